# revision 9
# baseline (speedup 1.0000x reference)
"""Trainium2 Bass kernel for nn_AUTOGCNLayer (GCN layer with two message-passing
rounds, three weight branches and mutual sigmoid gating).

Strategy (8 NeuronCores, node-parallel):
  - Nodes are sharded contiguously: core c owns rows [c*6250, (c+1)*6250),
    padded to 6272 = 49*128 rows per core.
  - Host-side preprocessing (index bookkeeping only): per core, edges are
    bucketed by destination window (128 nodes), split into lo/hi halves by
    source row (int16 gather index limit), padded to 128-edge chunks with a
    schedule that is identical across cores (SPMD single NEFF).
  - Round 1 gathers per-edge source rows DIRECTLY from the full padded f32
    feature table (replicated input, no collective): dma_gather with 512B
    descriptors; a per-chunk scalar-engine activation fuses the f32->bf16
    convert with the norm[src] scaling (per-edge scalar from host-shipped
    deg[src] ints).
  - Scatter-accumulate: one-hot matrices (iota == dstoff), built 8 chunks at
    a time in one DVE op via stride-0 broadcast APs, are the stationary
    matmul operands accumulating each 128-edge chunk into its PSUM window.
  - Round 1 output (norm^2-scaled, bf16) is AllGather'ed (the only
    collective), then round 2 repeats the gather/scatter from that table.
  - Epilogue per window: u = a*h + b*x combos, PE transpose, 128x128 weight
    matmuls in transposed layout, mutual sigmoid gating, bias, snorm, relu.
"""

import numpy as np

N = 50000
E = 600000
D = 128
KG = 8
EPS = 1e-09
NCORES = 8
NPC = N // NCORES            # 6250 real nodes per core
WINDOWS = (NPC + 127) // 128  # 49
NPAD = WINDOWS * 128          # 6272 padded nodes per core
SPLIT = 32768                 # lo/hi source split (int16 gather index limit)
AGROWS = NCORES * NPAD        # 50176 rows in the padded global tables
NQUEUES = 4
OHG = 8                       # chunks per batched one-hot build


# ---------------------------------------------------------------------------
# Host-side graph preprocessing (pure index bookkeeping)
# ---------------------------------------------------------------------------

def _preprocess(src, dst, max_call=8):
    """Build the shared chunk schedule and per-core index tensors.

    Returns (schedule, per_core) where schedule is shared across cores:
      chunk_wh: list of (window, half) per chunk slot
      calls:    list of (chunk_start, n_chunks, half) per dma_gather call
    and per_core[c] has:
      deg:    [128, WINDOWS] int32 in-degree (padded nodes 0)
      dstoff: [128, C_total] float32 (pad slots -1.0)
      degsrc: [128, C_total] int32 in-degree of each slot's source (pads 0)
      gidx:   [128, C_total*8] int16 gather indices in dma_gather layout
    """
    src = np.asarray(src, dtype=np.int64)
    dst = np.asarray(dst, dtype=np.int64)
    core = dst // NPC
    ldst = dst - core * NPC
    win = ldst // 128
    woff = ldst - win * 128
    # padded global source row (padded table indexing)
    psrc = (src // NPC) * NPAD + (src % NPC)
    half = (psrc >= SPLIT).astype(np.int64)

    counts = np.zeros((NCORES, WINDOWS, 2), dtype=np.int64)
    np.add.at(counts, (core, win, half), 1)
    kchunks = (counts + 127) // 128          # chunks per (core, window, half)
    Kwh = kchunks.max(axis=0)                 # shared schedule: [WINDOWS, 2]

    chunk_wh = []
    for w in range(WINDOWS):
        for h in (0, 1):
            chunk_wh.extend([(w, h)] * int(Kwh[w, h]))
    C_total = len(chunk_wh)

    # call list: contiguous same-half chunk runs, <= max_call each
    calls = []
    c0 = 0
    while c0 < C_total:
        h = chunk_wh[c0][1]
        c1 = c0
        while c1 < C_total and chunk_wh[c1][1] == h and c1 - c0 < max_call:
            c1 += 1
        calls.append((c0, c1 - c0, h))
        c0 = c1

    # block start offsets in the padded edge array follow the chunk order
    block_start = np.zeros((WINDOWS, 2), dtype=np.int64)
    seen = set()
    for ci, (w, h) in enumerate(chunk_wh):
        if (w, h) not in seen:
            seen.add((w, h))
            block_start[w, h] = ci * 128
    total_slots = C_total * 128

    # in-degree per node (counting-sort byproduct)
    deg_all = np.zeros(NCORES * NPC, dtype=np.int64)
    np.add.at(deg_all, dst, 1)

    per_core = []
    for c in range(NCORES):
        m = core == c
        w_c, h_c, off_c = win[m], half[m], woff[m]
        psrc_c, srcid_c = psrc[m], src[m]
        # stable ordering by (window, half) via counting offsets
        order = np.lexsort((psrc_c, h_c, w_c))
        w_c, h_c, off_c = w_c[order], h_c[order], off_c[order]
        psrc_c, srcid_c = psrc_c[order], srcid_c[order]
        key = w_c * 2 + h_c
        starts = np.searchsorted(key, np.arange(WINDOWS * 2))
        rank = np.arange(len(key)) - starts[key]
        slot = block_start[w_c, h_c] + rank

        dstoff_full = np.full(total_slots, -1.0, dtype=np.float32)
        gsrc_full = np.zeros(total_slots, dtype=np.int64)
        degsrc_full = np.zeros(total_slots, dtype=np.int32)
        dstoff_full[slot] = off_c.astype(np.float32)
        gsrc_full[slot] = psrc_c - h_c * SPLIT  # rebase hi half
        degsrc_full[slot] = deg_all[srcid_c]
        assert gsrc_full.max(initial=0) < SPLIT

        # dstoff layout [128, C_total]: slot s = chunk s//128, partition s%128
        dstoff_arr = dstoff_full.reshape(C_total, 128).T.copy()
        degsrc_arr = degsrc_full.reshape(C_total, 128).T.copy()

        # gather idx layout per call: linear idx i (over the call's slots)
        # lives at partition i%16 (replicated x8), free col call_base + i//16
        gidx_arr = np.zeros((128, C_total * 8), dtype=np.int16)
        for (cs, nch, _h) in calls:
            lin = gsrc_full[cs * 128:(cs + nch) * 128].astype(np.int16)
            wrapped = lin.reshape(nch * 8, 16).T  # [16, nch*8]
            gidx_arr[:, cs * 8:(cs + nch) * 8] = np.tile(wrapped, (8, 1))

        deg_c = np.zeros(NPAD, dtype=np.int32)
        deg_c[:NPC] = deg_all[c * NPC:(c + 1) * NPC]
        deg_arr = deg_c.reshape(WINDOWS, 128).T.copy()  # [128, WINDOWS]

        per_core.append({"deg": deg_arr, "dstoff": dstoff_arr,
                         "degsrc": degsrc_arr, "gidx": gidx_arr})

    return {"chunk_wh": chunk_wh, "calls": calls, "C_total": C_total}, per_core


# ---------------------------------------------------------------------------
# Bass kernel builder
# ---------------------------------------------------------------------------

def _build(schedule, epi_interleave=False, msgf_bufs=10, msg_bufs=16, oh_bufs=5,
           cvt_bufs=8, single_packet=True, shared_ag=False):
    import concourse.bacc as bacc
    import concourse.mybir as mybir
    import concourse.tile as tile
    from concourse.masks import make_identity

    chunk_wh = schedule["chunk_wh"]
    calls = schedule["calls"]
    C_total = schedule["C_total"]
    max_nch = max(nch for _, nch, _ in calls)
    f32 = mybir.dt.float32
    bf16 = mybir.dt.bfloat16
    edt = bf16
    AF = mybir.ActivationFunctionType
    OP = mybir.AluOpType

    nc = bacc.Bacc("TRN2", debug=False, num_swdge_queues=NQUEUES)

    feat_in = nc.dram_tensor("feature", [NPAD, D], f32, kind="ExternalInput")
    feattab_in = nc.dram_tensor("feattab", [AGROWS, D], f32, kind="ExternalInput")
    snorm_in = nc.dram_tensor("snorm", [128, WINDOWS], f32, kind="ExternalInput")
    deg_in = nc.dram_tensor("deg", [128, WINDOWS], mybir.dt.int32, kind="ExternalInput")
    degsrc_in = nc.dram_tensor("degsrc", [128, C_total], mybir.dt.int32, kind="ExternalInput")
    dstoff_in = nc.dram_tensor("dstoff", [128, C_total], f32, kind="ExternalInput")
    gidx_in = nc.dram_tensor("gidx", [128, C_total * 8], mybir.dt.int16, kind="ExternalInput")
    wlT_in = nc.dram_tensor("W_lowT", [D, D], f32, kind="ExternalInput")
    whT_in = nc.dram_tensor("W_highT", [D, D], f32, kind="ExternalInput")
    wmT_in = nc.dram_tensor("W_midT", [D, D], f32, kind="ExternalInput")
    gl_in = nc.dram_tensor("gamma_low", [1, KG], f32, kind="ExternalInput")
    gh_in = nc.dram_tensor("gamma_high", [1, KG], f32, kind="ExternalInput")
    gm_in = nc.dram_tensor("gamma_mid", [1, KG], f32, kind="ExternalInput")
    bias_in = nc.dram_tensor("bias", [128, 1], f32, kind="ExternalInput")
    out_dram = nc.dram_tensor("out", [NPAD, D], f32, kind="ExternalOutput")

    alpha = np.linspace(EPS, 1.0 - EPS, KG)
    midalpha = np.linspace(EPS, 1.0, KG)

    with tile.TileContext(nc) as tc:
        with (
            tc.tile_pool(name="const", bufs=1) as constp,
            tc.tile_pool(name="big", bufs=1) as bigp,
            tc.tile_pool(name="msg", bufs=1) as msgp,
            tc.tile_pool(name="oh", bufs=oh_bufs) as ohp,
            tc.tile_pool(name="cvt", bufs=cvt_bufs) as cvtp,
            tc.tile_pool(name="wrk", bufs=4) as wrkp,
            tc.tile_pool(name="pswin", bufs=3, space="PSUM") as pswin,
            tc.tile_pool(name="psep", bufs=2, space="PSUM") as psep,
            tc.tile_pool(name="dram", bufs=1, space="DRAM") as dramp,
        ):
            # ---------------- constants / small tiles ----------------
            iota_i = constp.tile([128, 128], mybir.dt.int32)
            nc.gpsimd.iota(iota_i[:], pattern=[[1, 128]], base=0, channel_multiplier=0)
            iota_f = constp.tile([128, 128], f32)
            nc.vector.tensor_copy(iota_f[:], iota_i[:])
            ident = constp.tile([128, 128], edt)
            make_identity(nc, ident[:])

            dstoff_t = constp.tile([128, C_total], f32)
            nc.sync.dma_start(dstoff_t[:], dstoff_in[:])
            gidx_t = constp.tile([128, C_total * 8], mybir.dt.int16)
            nc.sync.dma_start(gidx_t[:], gidx_in[:])
            snorm_t = constp.tile([128, WINDOWS], f32)
            nc.sync.dma_start(snorm_t[:], snorm_in[:])
            bias_t = constp.tile([128, 1], f32)
            nc.sync.dma_start(bias_t[:], bias_in[:])
            wT = {}
            for nm, drt in (("low", wlT_in), ("high", whT_in), ("mid", wmT_in)):
                t32 = constp.tile([128, 128], f32, tag=f"w32{nm}")
                nc.sync.dma_start(t32[:], drt[:])
                t = constp.tile([128, 128], edt, tag=f"w{nm}")
                nc.vector.tensor_copy(t[:], t32[:])
                wT[nm] = t

            # deg -> norm, norm^2   [128, WINDOWS]
            deg_t = constp.tile([128, WINDOWS], mybir.dt.int32)
            nc.sync.dma_start(deg_t[:], deg_in[:])
            deg_f = constp.tile([128, WINDOWS], f32)
            nc.vector.tensor_copy(deg_f[:], deg_t[:])
            nc.vector.tensor_scalar_max(deg_f[:], deg_f[:], 1.0)
            sq = constp.tile([128, WINDOWS], f32)
            nc.scalar.activation(sq[:], deg_f[:], AF.Sqrt)
            norm_t = constp.tile([128, WINDOWS], f32)
            nc.vector.reciprocal(norm_t[:], sq[:])
            norm2_t = constp.tile([128, WINDOWS], f32)
            nc.vector.tensor_tensor(norm2_t[:], norm_t[:], norm_t[:], OP.mult)

            # degsrc -> normsrc  [128, C_total]
            degsrc_t = constp.tile([128, C_total], mybir.dt.int32)
            nc.sync.dma_start(degsrc_t[:], degsrc_in[:])
            degsrc_f = constp.tile([128, C_total], f32)
            nc.vector.tensor_copy(degsrc_f[:], degsrc_t[:])
            nc.vector.tensor_scalar_max(degsrc_f[:], degsrc_f[:], 1.0)
            sqs = constp.tile([128, C_total], f32)
            nc.scalar.activation(sqs[:], degsrc_f[:], AF.Sqrt)
            normsrc_t = constp.tile([128, C_total], f32)
            nc.vector.reciprocal(normsrc_t[:], sqs[:])

            # ---------------- epilogue scalar coefficients ----------------
            ones_row = constp.tile([1, 128], f32)
            nc.vector.memset(ones_row[:], 1.0)
            coeff = {}
            for nm, drt in (("low", gl_in), ("high", gh_in), ("mid", gm_in)):
                g_small = constp.tile([1, KG], f32, tag=f"gs{nm}")
                nc.sync.dma_start(g_small[:], drt[:])
                g_ps = psep.tile([128, KG], f32, space="PSUM", tag="uT")
                nc.tensor.matmul(g_ps[:], lhsT=ones_row[:], rhs=g_small[:],
                                 start=True, stop=True)
                g_b = constp.tile([128, KG], f32, tag=f"gb{nm}")
                nc.scalar.activation(g_b[:], g_ps[:], AF.Relu)
                coeff[nm] = g_b

            def dotcol(gb, weights, tag):
                wt = constp.tile([128, KG], f32, tag=f"wt{tag}")
                for i, v in enumerate(weights):
                    nc.vector.memset(wt[:, i:i + 1], float(v))
                prod = constp.tile([128, KG], f32, tag=f"pr{tag}")
                nc.vector.tensor_tensor(prod[:], gb[:], wt[:], OP.mult)
                col = constp.tile([128, 1], f32, tag=f"col{tag}")
                nc.vector.tensor_reduce(col[:], prod[:], mybir.AxisListType.X, OP.add)
                return col

            a0_col = dotcol(coeff["low"], alpha, "a0")
            b0_col = dotcol(coeff["low"], 1.0 - alpha, "b0")
            a1p_col = dotcol(coeff["high"], alpha, "a1p")   # positive; negated below
            b1_col = dotcol(coeff["high"], 1.0 - alpha, "b1")
            c2_col = dotcol(coeff["mid"], np.ones(KG), "c2")
            d2_col = dotcol(coeff["mid"], midalpha, "d2")
            a1_col = constp.tile([128, 1], f32)
            nc.vector.tensor_scalar_mul(a1_col[:], a1p_col[:], -1.0)

            # ---------------- load feature tiles ----------------
            with nc.named_scope("xload"):
                x_buf = bigp.tile([128, NPAD], edt)   # window w at [:, w*128:(w+1)*128]
                nc.gpsimd.dma_start(
                    x_buf[:].rearrange("p (w d) -> p w d", d=D),
                    feat_in[:].rearrange("(w p) d -> p w d", p=128),
                )

            h_buf = bigp.tile([128, NPAD], edt)
            h1_buf = bigp.tile([128, NPAD], edt)
            nh_buf = bigp.tile([128, NPAD], bf16, tag="nh_buf")
            bounce2 = dramp.tile([NPAD, D], bf16)
            agkw = dict(addr_space="Shared") if shared_ag else {}
            ag2 = dramp.tile([AGROWS, D], bf16, **agkw)

            first_chunk = {}
            last_chunk = {}
            for c0, (w, h) in enumerate(chunk_wh):
                first_chunk.setdefault(w, c0)
                last_chunk[w] = c0

            def mp_round(rnd, out_h_buf, write_normh, window_done=None):
                """One message-passing round; rnd 1 gathers f32 from feattab,
                rnd 2 gathers bf16 from ag2."""
                msg_tiles = {}
                for ci, (cs, nch, h) in enumerate(calls):
                    if rnd == 1:
                        mt = msgp.tile([128, max_nch, 128], f32, tag="msgf",
                                       bufs=msgf_bufs)
                        base = feattab_in[SPLIT:, :] if h else feattab_in[:SPLIT, :]
                    else:
                        mt = msgp.tile([128, max_nch, 128], bf16, tag="msg",
                                       bufs=msg_bufs)
                        base = ag2[SPLIT:, :] if h else ag2[:SPLIT, :]
                    num_idxs = nch * 128
                    nc.gpsimd.dma_gather(
                        mt[:, :nch, :], base, gidx_t[:, cs * 8:cs * 8 + nch * 8],
                        num_idxs, num_idxs, D, queue_num=ci % NQUEUES,
                        single_packet=single_packet,
                    )
                    for k in range(nch):
                        msg_tiles[cs + k] = (mt, k)

                # batched one-hot builds + chunk->window accumulation
                oh_tiles = {}
                open_psum = {}
                for c0, (w, h) in enumerate(chunk_wh):
                    g0 = c0 - (c0 % OHG)
                    if c0 == g0:
                        g = min(OHG, C_total - g0)
                        oh = ohp.tile([128, OHG, 128], bf16, tag="oh")
                        nc.vector.tensor_tensor(
                            oh[:, :g, :],
                            iota_f[:].rearrange("p (g d) -> p g d", g=1)
                                     .to_broadcast([128, g, 128]),
                            dstoff_t[:, g0:g0 + g]
                                     .rearrange("p (g o) -> p g o", o=1)
                                     .to_broadcast([128, g, 128]),
                            OP.is_equal)
                        oh_tiles[g0] = oh
                    if c0 == first_chunk[w]:
                        psw_new = pswin.tile([128, 128], f32, space="PSUM", tag="agg")
                        open_psum[w] = psw_new
                    psum_w = open_psum[w]
                    mt, k = msg_tiles[c0]
                    if rnd == 1:
                        rhs = cvtp.tile([128, 128], bf16, tag="cvt")
                        nc.scalar.activation(rhs[:], mt[:, k, :], AF.Copy,
                                             scale=normsrc_t[:, c0:c0 + 1])
                    else:
                        rhs = mt[:, k, :]
                    last = c0 == last_chunk[w]
                    nc.tensor.matmul(psum_w[:], lhsT=oh_tiles[g0][:, c0 - g0, :],
                                     rhs=rhs,
                                     start=c0 == first_chunk[w], stop=last)
                    if last:
                        del open_psum[w]
                        nc.scalar.activation(
                            out_h_buf[:, w * 128:(w + 1) * 128], psum_w[:],
                            AF.Copy, scale=norm_t[:, w:w + 1])
                        if write_normh:
                            nc.scalar.activation(
                                nh_buf[:, w * 128:(w + 1) * 128], psum_w[:],
                                AF.Copy, scale=norm2_t[:, w:w + 1])
                        if window_done is not None:
                            window_done(w)

            def epilogue_window(w):
                sl = slice(w * 128, (w + 1) * 128)
                x_w = x_buf[:, sl]
                h_w = h_buf[:, sl]
                h1_w = h1_buf[:, sl]

                def combo(in_hi, a_col, x_col, op1, tag):
                    xb = wrkp.tile([128, 128], edt, tag=f"xb{tag}")
                    nc.vector.tensor_scalar_mul(xb[:], x_w, x_col[:])
                    u = wrkp.tile([128, 128], edt, tag=f"u{tag}")
                    nc.vector.scalar_tensor_tensor(
                        out=u[:], in0=in_hi, scalar=a_col[:], in1=xb[:],
                        op0=OP.mult, op1=op1)
                    return u

                u0 = combo(h_w, a0_col, b0_col, OP.add, "0")
                u1 = combo(h_w, a1_col, b1_col, OP.add, "1")
                u2 = combo(h1_w, c2_col, d2_col, OP.subtract, "2")

                oT = {}
                for nm, u in (("low", u0), ("high", u1), ("mid", u2)):
                    up = psep.tile([128, 128], edt, space="PSUM", tag="uT")
                    nc.tensor.transpose(up[:], u[:], ident[:])
                    uT = wrkp.tile([128, 128], edt, tag=f"uT{nm}")
                    nc.vector.tensor_copy(uT[:], up[:])
                    op = psep.tile([128, 128], f32, space="PSUM", tag="om")
                    nc.tensor.matmul(op[:], lhsT=wT[nm][:], rhs=uT[:],
                                     start=True, stop=True)
                    ot = wrkp.tile([128, 128], edt, tag=f"ot{nm}")
                    nc.scalar.copy(ot[:], op[:])
                    oT[nm] = ot

                # mutual gating (T layout)
                tmp = wrkp.tile([128, 128], edt, tag="gt")
                sig = wrkp.tile([128, 128], edt, tag="gs")
                nc.vector.tensor_tensor(tmp[:], oT["high"][:], oT["mid"][:], OP.add)
                nc.scalar.activation(sig[:], tmp[:], AF.Sigmoid)
                nc.vector.tensor_tensor(oT["low"][:], oT["low"][:], sig[:], OP.mult)
                nc.vector.tensor_tensor(tmp[:], oT["low"][:], oT["mid"][:], OP.add)
                nc.scalar.activation(sig[:], tmp[:], AF.Sigmoid)
                nc.vector.tensor_tensor(oT["high"][:], oT["high"][:], sig[:], OP.mult)
                nc.vector.tensor_tensor(tmp[:], oT["low"][:], oT["high"][:], OP.add)
                nc.scalar.activation(sig[:], tmp[:], AF.Sigmoid)
                nc.vector.tensor_tensor(oT["mid"][:], oT["mid"][:], sig[:], OP.mult)

                nc.vector.tensor_tensor(tmp[:], oT["low"][:], oT["high"][:], OP.add)
                nc.vector.tensor_tensor(tmp[:], tmp[:], oT["mid"][:], OP.add)
                nc.vector.tensor_scalar_add(tmp[:], tmp[:], bias_t[:])

                # back to row layout; relu(x * snorm)
                bp = psep.tile([128, 128], edt, space="PSUM", tag="uT")
                nc.tensor.transpose(bp[:], tmp[:], ident[:])
                outt = wrkp.tile([128, 128], f32, tag="outt")
                nc.scalar.activation(outt[:], bp[:], AF.Relu,
                                     scale=snorm_t[:, w:w + 1])
                nc.sync.dma_start(out_dram[w * 128:(w + 1) * 128, :], outt[:])

            def epilogue_window_scoped(w):
                with nc.named_scope("epi"):
                    epilogue_window(w)

            with nc.named_scope("mp1"):
                mp_round(1, h_buf, write_normh=True)
            with nc.named_scope("ag2"):
                nc.sync.dma_start(
                    bounce2[:].rearrange("(w p) d -> p w d", p=128),
                    nh_buf[:].rearrange("p (w d) -> p w d", d=D))
                nc.gpsimd.collective_compute(
                    "AllGather", mybir.AluOpType.bypass,
                    ins=[bounce2.opt()], outs=[ag2.opt()],
                    replica_groups=[list(range(NCORES))],
                )
            if epi_interleave:
                with nc.named_scope("mp2"):
                    mp_round(2, h1_buf, write_normh=False,
                             window_done=epilogue_window_scoped)
            else:
                with nc.named_scope("mp2"):
                    mp_round(2, h1_buf, write_normh=False)
                for w in range(WINDOWS):
                    epilogue_window_scoped(w)

    nc.compile()
    return nc


# ---------------------------------------------------------------------------
# Public entry point
# ---------------------------------------------------------------------------

def build_and_inputs(feature, snorm_n, src, dst, W_low, W_high, W_mid,
                     gamma_low, gamma_high, gamma_mid, bias,
                     max_call=8, **build_kwargs):
    feature = np.asarray(feature, dtype=np.float32)
    snorm_n = np.asarray(snorm_n, dtype=np.float32)
    schedule, per_core = _preprocess(np.asarray(src), np.asarray(dst),
                                     max_call=max_call)
    nc = _build(schedule, **build_kwargs)

    feattab = np.zeros((AGROWS, D), np.float32)
    for c in range(NCORES):
        feattab[c * NPAD:c * NPAD + NPC] = feature[c * NPC:(c + 1) * NPC]

    in_maps = []
    for c in range(NCORES):
        feat_c = feattab[c * NPAD:(c + 1) * NPAD]
        sn_c = np.zeros(NPAD, np.float32)
        sn_c[:NPC] = snorm_n[c * NPC:(c + 1) * NPC, 0]
        in_maps.append({
            "feature": feat_c,
            "feattab": feattab,
            "snorm": sn_c.reshape(WINDOWS, 128).T.copy(),
            "deg": per_core[c]["deg"],
            "degsrc": per_core[c]["degsrc"],
            "dstoff": per_core[c]["dstoff"],
            "gidx": per_core[c]["gidx"],
            "W_lowT": np.ascontiguousarray(np.asarray(W_low, np.float32).T),
            "W_highT": np.ascontiguousarray(np.asarray(W_high, np.float32).T),
            "W_midT": np.ascontiguousarray(np.asarray(W_mid, np.float32).T),
            "gamma_low": np.asarray(gamma_low, np.float32).reshape(1, KG),
            "gamma_high": np.asarray(gamma_high, np.float32).reshape(1, KG),
            "gamma_mid": np.asarray(gamma_mid, np.float32).reshape(1, KG),
            "bias": np.asarray(bias, np.float32).reshape(128, 1),
        })

    return nc, in_maps


def kernel(**inputs):
    from concourse.bass_utils import run_bass_kernel_spmd

    nc, in_maps = build_and_inputs(**inputs)
    res = run_bass_kernel_spmd(nc, in_maps, core_ids=list(range(NCORES)))
    out = np.concatenate(
        [res.results[c]["out"][:NPC] for c in range(NCORES)], axis=0)
    return out


# revision 24
# speedup vs baseline: 1.1797x; 1.1797x over previous
"""Trainium2 Bass kernel for nn_AUTOGCNLayer (GCN layer with two message-passing
rounds, three weight branches and mutual sigmoid gating).

Strategy (8 NeuronCores, node-parallel):
  - Nodes are sharded contiguously: core c owns rows [c*6250, (c+1)*6250),
    padded to 6272 = 49*128 rows per core.
  - Host-side preprocessing (index bookkeeping only): per core, edges are
    bucketed by destination window (128 nodes), split into lo/hi halves by
    source row (int16 gather index limit), padded to 128-edge chunks with a
    schedule that is identical across cores (SPMD single NEFF).
  - Round 1 gathers per-edge source rows DIRECTLY from the full padded f32
    feature table (replicated input, no collective): dma_gather with 512B
    descriptors; a per-chunk scalar-engine activation fuses the f32->bf16
    convert with the norm[src] scaling (per-edge scalar from host-shipped
    deg[src] ints).
  - Scatter-accumulate: one-hot matrices (iota == dstoff), built 8 chunks at
    a time in one DVE op via stride-0 broadcast APs, are the stationary
    matmul operands accumulating each 128-edge chunk into its PSUM window.
  - Round 1 output (norm^2-scaled, bf16) is AllGather'ed (the only
    collective), then round 2 repeats the gather/scatter from that table.
  - Epilogue per window: u = a*h + b*x combos, PE transpose, 128x128 weight
    matmuls in transposed layout, mutual sigmoid gating, bias, snorm, relu.
"""

import numpy as np

N = 50000
E = 600000
D = 128
KG = 8
EPS = 1e-09
NCORES = 8
NPC = N // NCORES            # 6250 real nodes per core
WINDOWS = (NPC + 127) // 128  # 49
NPAD = WINDOWS * 128          # 6272 padded nodes per core
SPLIT = 32768                 # lo/hi source split (int16 gather index limit)
AGROWS = NCORES * NPAD        # 50176 rows in the padded global tables
NQUEUES = 4
OHG = 8                       # chunks per batched one-hot build


# ---------------------------------------------------------------------------
# Host-side graph preprocessing (pure index bookkeeping)
# ---------------------------------------------------------------------------

def _preprocess(src, dst, max_call=8, trim_pads=True):
    """Build the shared chunk schedule and per-core index tensors.

    Returns (schedule, per_core) where schedule is shared across cores:
      chunk_wh: list of (window, half) per chunk slot
      calls:    list of (chunk_start, n_chunks, half) per dma_gather call
    and per_core[c] has:
      deg:    [128, WINDOWS] int32 in-degree (padded nodes 0)
      dstoff: [128, C_total] float32 (pad slots -1.0)
      degsrc: [128, C_total] int32 in-degree of each slot's source (pads 0)
      gidx:   [128, C_total*8] int16 gather indices in dma_gather layout
    """
    src = np.asarray(src, dtype=np.int64)
    dst = np.asarray(dst, dtype=np.int64)
    core = dst // NPC
    ldst = dst - core * NPC
    win = ldst // 128
    woff = ldst - win * 128
    # padded global source row (padded table indexing)
    psrc = (src // NPC) * NPAD + (src % NPC)
    half = (psrc >= SPLIT).astype(np.int64)

    counts = np.zeros((NCORES, WINDOWS, 2), dtype=np.int64)
    np.add.at(counts, (core, win, half), 1)
    kchunks = (counts + 127) // 128          # chunks per (core, window, half)
    Kwh = kchunks.max(axis=0)                 # shared schedule: [WINDOWS, 2]

    chunk_wh = []
    for w in range(WINDOWS):
        for h in (0, 1):
            chunk_wh.extend([(w, h)] * int(Kwh[w, h]))
    C_total = len(chunk_wh)

    # call list: <= max_call chunks each, never crossing a (window, half)
    # block boundary so that pad slots (gather idx -1) are trailing within
    # their call and get trimmed by the Q7 desc-gen kernel
    calls = []
    c0 = 0
    while c0 < C_total:
        w, h = chunk_wh[c0]
        c1 = c0
        while c1 < C_total and chunk_wh[c1] == (w, h):
            c1 += 1
        b = c0
        while b < c1:
            n = min(max_call, c1 - b)
            calls.append((b, n, h))
            b += n
        c0 = c1

    # block start offsets in the padded edge array follow the chunk order
    block_start = np.zeros((WINDOWS, 2), dtype=np.int64)
    seen = set()
    for ci, (w, h) in enumerate(chunk_wh):
        if (w, h) not in seen:
            seen.add((w, h))
            block_start[w, h] = ci * 128
    total_slots = C_total * 128

    # in-degree per node (counting-sort byproduct)
    deg_all = np.zeros(NCORES * NPC, dtype=np.int64)
    np.add.at(deg_all, dst, 1)

    per_core = []
    for c in range(NCORES):
        m = core == c
        w_c, h_c, off_c = win[m], half[m], woff[m]
        psrc_c, srcid_c = psrc[m], src[m]
        # stable ordering by (window, half) via counting offsets
        order = np.lexsort((psrc_c, h_c, w_c))
        w_c, h_c, off_c = w_c[order], h_c[order], off_c[order]
        psrc_c, srcid_c = psrc_c[order], srcid_c[order]
        key = w_c * 2 + h_c
        starts = np.searchsorted(key, np.arange(WINDOWS * 2))
        rank = np.arange(len(key)) - starts[key]
        slot = block_start[w_c, h_c] + rank

        dstoff_full = np.full(total_slots, -1.0, dtype=np.float32)
        gsrc_full = np.full(total_slots, -1 if trim_pads else 0,
                            dtype=np.int64)  # -1 pads are trimmed by ucode
        degsrc_full = np.zeros(total_slots, dtype=np.int32)
        dstoff_full[slot] = off_c.astype(np.float32)
        gsrc_full[slot] = psrc_c - h_c * SPLIT  # rebase hi half
        degsrc_full[slot] = deg_all[srcid_c]
        assert gsrc_full.max(initial=0) < SPLIT

        # dstoff layout [128, C_total]: slot s = chunk s//128, partition s%128
        dstoff_arr = dstoff_full.reshape(C_total, 128).T.copy()
        degsrc_arr = degsrc_full.reshape(C_total, 128).T.copy()

        # gather idx layout per call: linear idx i (over the call's slots)
        # lives at partition i%16 (replicated x8), free col call_base + i//16
        gidx_arr = np.zeros((128, C_total * 8), dtype=np.int16)
        for (cs, nch, _h) in calls:
            lin = gsrc_full[cs * 128:(cs + nch) * 128].astype(np.int16)
            wrapped = lin.reshape(nch * 8, 16).T  # [16, nch*8]
            gidx_arr[:, cs * 8:(cs + nch) * 8] = np.tile(wrapped, (8, 1))

        deg_c = np.zeros(NPAD, dtype=np.int32)
        deg_c[:NPC] = deg_all[c * NPC:(c + 1) * NPC]
        deg_arr = deg_c.reshape(WINDOWS, 128).T.copy()  # [128, WINDOWS]

        per_core.append({"deg": deg_arr, "dstoff": dstoff_arr,
                         "degsrc": degsrc_arr, "gidx": gidx_arr})

    return {"chunk_wh": chunk_wh, "calls": calls, "C_total": C_total}, per_core


# ---------------------------------------------------------------------------
# Bass kernel builder
# ---------------------------------------------------------------------------

def _build(schedule, epi_interleave=False, msgf_bufs=10, msg_bufs=16, oh_bufs=5,
           cvt_bufs=8, single_packet=True, shared_ag=False, psep_bufs=2,
           wrk_bufs=4, pswin_bufs=3, cvt_mix=0):
    import concourse.bacc as bacc
    import concourse.mybir as mybir
    import concourse.tile as tile
    from concourse.masks import make_identity

    chunk_wh = schedule["chunk_wh"]
    calls = schedule["calls"]
    C_total = schedule["C_total"]
    max_nch = max(nch for _, nch, _ in calls)
    f32 = mybir.dt.float32
    bf16 = mybir.dt.bfloat16
    edt = bf16
    AF = mybir.ActivationFunctionType
    OP = mybir.AluOpType

    nc = bacc.Bacc("TRN2", debug=False, num_swdge_queues=NQUEUES)

    feat_in = nc.dram_tensor("feature", [NPAD, D], f32, kind="ExternalInput")
    feattab_in = nc.dram_tensor("feattab", [AGROWS, D], f32, kind="ExternalInput")
    snorm_in = nc.dram_tensor("snorm", [128, WINDOWS], f32, kind="ExternalInput")
    deg_in = nc.dram_tensor("deg", [128, WINDOWS], mybir.dt.int32, kind="ExternalInput")
    degsrc_in = nc.dram_tensor("degsrc", [128, C_total], mybir.dt.int32, kind="ExternalInput")
    dstoff_in = nc.dram_tensor("dstoff", [128, C_total], f32, kind="ExternalInput")
    gidx_in = nc.dram_tensor("gidx", [128, C_total * 8], mybir.dt.int16, kind="ExternalInput")
    wlT_in = nc.dram_tensor("W_lowT", [D, D], f32, kind="ExternalInput")
    whT_in = nc.dram_tensor("W_highT", [D, D], f32, kind="ExternalInput")
    wmT_in = nc.dram_tensor("W_midT", [D, D], f32, kind="ExternalInput")
    gl_in = nc.dram_tensor("gamma_low", [1, KG], f32, kind="ExternalInput")
    gh_in = nc.dram_tensor("gamma_high", [1, KG], f32, kind="ExternalInput")
    gm_in = nc.dram_tensor("gamma_mid", [1, KG], f32, kind="ExternalInput")
    bias_in = nc.dram_tensor("bias", [128, 1], f32, kind="ExternalInput")
    out_dram = nc.dram_tensor("out", [NPAD, D], f32, kind="ExternalOutput")

    alpha = np.linspace(EPS, 1.0 - EPS, KG)
    midalpha = np.linspace(EPS, 1.0, KG)

    with tile.TileContext(nc) as tc:
        with (
            tc.tile_pool(name="const", bufs=1) as constp,
            tc.tile_pool(name="big", bufs=1) as bigp,
            tc.tile_pool(name="msg", bufs=1) as msgp,
            tc.tile_pool(name="oh", bufs=oh_bufs) as ohp,
            tc.tile_pool(name="cvt", bufs=cvt_bufs) as cvtp,
            tc.tile_pool(name="wrk", bufs=wrk_bufs) as wrkp,
            tc.tile_pool(name="pswin", bufs=pswin_bufs, space="PSUM") as pswin,
            tc.tile_pool(name="psep", bufs=psep_bufs, space="PSUM") as psep,
            tc.tile_pool(name="dram", bufs=1, space="DRAM") as dramp,
        ):
            # ---------------- constants / small tiles ----------------
            iota_i = constp.tile([128, 128], mybir.dt.int32)
            nc.gpsimd.iota(iota_i[:], pattern=[[1, 128]], base=0, channel_multiplier=0)
            iota_f = constp.tile([128, 128], f32)
            nc.vector.tensor_copy(iota_f[:], iota_i[:])
            ident = constp.tile([128, 128], edt)
            make_identity(nc, ident[:])

            dstoff_t = constp.tile([128, C_total], f32)
            nc.sync.dma_start(dstoff_t[:], dstoff_in[:])
            gidx_t = constp.tile([128, C_total * 8], mybir.dt.int16)
            nc.sync.dma_start(gidx_t[:], gidx_in[:])
            snorm_t = constp.tile([128, WINDOWS], f32)
            nc.sync.dma_start(snorm_t[:], snorm_in[:])
            bias_t = constp.tile([128, 1], f32)
            nc.sync.dma_start(bias_t[:], bias_in[:])
            wT = {}
            for nm, drt in (("low", wlT_in), ("high", whT_in), ("mid", wmT_in)):
                t32 = constp.tile([128, 128], f32, tag=f"w32{nm}")
                nc.sync.dma_start(t32[:], drt[:])
                t = constp.tile([128, 128], edt, tag=f"w{nm}")
                nc.vector.tensor_copy(t[:], t32[:])
                wT[nm] = t

            # deg -> norm, norm^2   [128, WINDOWS]
            deg_t = constp.tile([128, WINDOWS], mybir.dt.int32)
            nc.sync.dma_start(deg_t[:], deg_in[:])
            deg_f = constp.tile([128, WINDOWS], f32)
            nc.vector.tensor_copy(deg_f[:], deg_t[:])
            nc.vector.tensor_scalar_max(deg_f[:], deg_f[:], 1.0)
            sq = constp.tile([128, WINDOWS], f32)
            nc.scalar.activation(sq[:], deg_f[:], AF.Sqrt)
            norm_t = constp.tile([128, WINDOWS], f32)
            nc.vector.reciprocal(norm_t[:], sq[:])
            norm2_t = constp.tile([128, WINDOWS], f32)
            nc.vector.tensor_tensor(norm2_t[:], norm_t[:], norm_t[:], OP.mult)

            # degsrc -> normsrc  [128, C_total]
            degsrc_t = constp.tile([128, C_total], mybir.dt.int32)
            nc.sync.dma_start(degsrc_t[:], degsrc_in[:])
            degsrc_f = constp.tile([128, C_total], f32)
            nc.vector.tensor_copy(degsrc_f[:], degsrc_t[:])
            nc.vector.tensor_scalar_max(degsrc_f[:], degsrc_f[:], 1.0)
            sqs = constp.tile([128, C_total], f32)
            nc.scalar.activation(sqs[:], degsrc_f[:], AF.Sqrt)
            normsrc_t = constp.tile([128, C_total], f32)
            nc.vector.reciprocal(normsrc_t[:], sqs[:])

            # ---------------- epilogue scalar coefficients ----------------
            ones_row = constp.tile([1, 128], f32)
            nc.vector.memset(ones_row[:], 1.0)
            coeff = {}
            for nm, drt in (("low", gl_in), ("high", gh_in), ("mid", gm_in)):
                g_small = constp.tile([1, KG], f32, tag=f"gs{nm}")
                nc.sync.dma_start(g_small[:], drt[:])
                g_ps = psep.tile([128, KG], f32, space="PSUM", tag="uT")
                nc.tensor.matmul(g_ps[:], lhsT=ones_row[:], rhs=g_small[:],
                                 start=True, stop=True)
                g_b = constp.tile([128, KG], f32, tag=f"gb{nm}")
                nc.scalar.activation(g_b[:], g_ps[:], AF.Relu)
                coeff[nm] = g_b

            def dotcol(gb, weights, tag):
                wt = constp.tile([128, KG], f32, tag=f"wt{tag}")
                for i, v in enumerate(weights):
                    nc.vector.memset(wt[:, i:i + 1], float(v))
                prod = constp.tile([128, KG], f32, tag=f"pr{tag}")
                nc.vector.tensor_tensor(prod[:], gb[:], wt[:], OP.mult)
                col = constp.tile([128, 1], f32, tag=f"col{tag}")
                nc.vector.tensor_reduce(col[:], prod[:], mybir.AxisListType.X, OP.add)
                return col

            a0_col = dotcol(coeff["low"], alpha, "a0")
            b0_col = dotcol(coeff["low"], 1.0 - alpha, "b0")
            a1p_col = dotcol(coeff["high"], alpha, "a1p")   # positive; negated below
            b1_col = dotcol(coeff["high"], 1.0 - alpha, "b1")
            c2_col = dotcol(coeff["mid"], np.ones(KG), "c2")
            d2_col = dotcol(coeff["mid"], midalpha, "d2")
            a1_col = constp.tile([128, 1], f32)
            nc.vector.tensor_scalar_mul(a1_col[:], a1p_col[:], -1.0)

            # ---------------- load feature tiles ----------------
            with nc.named_scope("xload"):
                x_buf = bigp.tile([128, NPAD], edt)   # window w at [:, w*128:(w+1)*128]
                nc.gpsimd.dma_start(
                    x_buf[:].rearrange("p (w d) -> p w d", d=D),
                    feat_in[:].rearrange("(w p) d -> p w d", p=128),
                )

            # pre-touch every msg pool buffer: pad slots with gather idx -1
            # are never written by the DMA, and stale uninitialized SBUF can
            # decode as NaN (0 * NaN = NaN would poison the PSUM accumulate)
            for i in range(msgf_bufs):
                mt0 = msgp.tile([128, max_nch, 128], f32, tag="msgf",
                                bufs=msgf_bufs, name=f"mi{i}")
                nc.vector.memset(mt0[:].rearrange("p c d -> p (c d)"), 0.0)
            for i in range(msg_bufs):
                mt0 = msgp.tile([128, max_nch, 128], bf16, tag="msg",
                                bufs=msg_bufs, name=f"mj{i}")
                nc.vector.memset(mt0[:].rearrange("p c d -> p (c d)"), 0.0)

            h_buf = bigp.tile([128, NPAD], edt)
            h1_buf = bigp.tile([128, NPAD], edt)
            nh_buf = bigp.tile([128, NPAD], bf16, tag="nh_buf")
            bounce2 = dramp.tile([NPAD, D], bf16)
            agkw = dict(addr_space="Shared") if shared_ag else {}
            ag2 = dramp.tile([AGROWS, D], bf16, **agkw)

            first_chunk = {}
            last_chunk = {}
            for c0, (w, h) in enumerate(chunk_wh):
                first_chunk.setdefault(w, c0)
                last_chunk[w] = c0

            def mp_round(rnd, out_h_buf, write_normh, window_done=None):
                """One message-passing round; rnd 1 gathers f32 from feattab,
                rnd 2 gathers bf16 from ag2."""
                msg_tiles = {}
                for ci, (cs, nch, h) in enumerate(calls):
                    if rnd == 1:
                        mt = msgp.tile([128, max_nch, 128], f32, tag="msgf",
                                       bufs=msgf_bufs)
                        base = feattab_in[SPLIT:, :] if h else feattab_in[:SPLIT, :]
                    else:
                        mt = msgp.tile([128, max_nch, 128], bf16, tag="msg",
                                       bufs=msg_bufs)
                        base = ag2[SPLIT:, :] if h else ag2[:SPLIT, :]
                    num_idxs = nch * 128
                    nc.gpsimd.dma_gather(
                        mt[:, :nch, :], base, gidx_t[:, cs * 8:cs * 8 + nch * 8],
                        num_idxs, num_idxs, D, queue_num=ci % NQUEUES,
                        single_packet=single_packet,
                    )
                    for k in range(nch):
                        msg_tiles[cs + k] = (mt, k)

                # batched one-hot builds + chunk->window accumulation
                oh_tiles = {}
                open_psum = {}
                for c0, (w, h) in enumerate(chunk_wh):
                    g0 = c0 - (c0 % OHG)
                    if c0 == g0:
                        g = min(OHG, C_total - g0)
                        oh = ohp.tile([128, OHG, 128], bf16, tag="oh")
                        nc.vector.tensor_tensor(
                            oh[:, :g, :],
                            iota_f[:].rearrange("p (g d) -> p g d", g=1)
                                     .to_broadcast([128, g, 128]),
                            dstoff_t[:, g0:g0 + g]
                                     .rearrange("p (g o) -> p g o", o=1)
                                     .to_broadcast([128, g, 128]),
                            OP.is_equal)
                        oh_tiles[g0] = oh
                    if c0 == first_chunk[w]:
                        psw_new = pswin.tile([128, 128], f32, space="PSUM", tag="agg")
                        open_psum[w] = psw_new
                    psum_w = open_psum[w]
                    mt, k = msg_tiles[c0]
                    if rnd == 1:
                        rhs = cvtp.tile([128, 128], bf16, tag="cvt")
                        if cvt_mix and c0 % cvt_mix == 0:
                            nc.vector.tensor_scalar_mul(
                                rhs[:], mt[:, k, :], normsrc_t[:, c0:c0 + 1])
                        else:
                            nc.scalar.activation(rhs[:], mt[:, k, :], AF.Copy,
                                                 scale=normsrc_t[:, c0:c0 + 1])
                    else:
                        rhs = mt[:, k, :]
                    last = c0 == last_chunk[w]
                    nc.tensor.matmul(psum_w[:], lhsT=oh_tiles[g0][:, c0 - g0, :],
                                     rhs=rhs,
                                     start=c0 == first_chunk[w], stop=last)
                    if last:
                        del open_psum[w]
                        nc.scalar.activation(
                            out_h_buf[:, w * 128:(w + 1) * 128], psum_w[:],
                            AF.Copy, scale=norm_t[:, w:w + 1])
                        if write_normh:
                            nc.scalar.activation(
                                nh_buf[:, w * 128:(w + 1) * 128], psum_w[:],
                                AF.Copy, scale=norm2_t[:, w:w + 1])
                        if window_done is not None:
                            window_done(w)

            # wide epilogue precomputation: u0/u1 need only h and x, and the
            # coefficient columns are constant across windows, so compute them
            # in a few whole-buffer DVE ops (tensor_scalar with a [128,1]
            # scalar AP costs ~1.1us per 128x128 window; wide ops amortize it)
            u0_buf = bigp.tile([128, NPAD], edt, tag="u0_buf")
            u1_buf = bigp.tile([128, NPAD], edt, tag="u1_buf")
            xb2_buf = bigp.tile([128, NPAD], edt, tag="xb2_buf")

            def wide_epi_pre():
                nc.vector.tensor_scalar_mul(u0_buf[:], x_buf[:], b0_col[:])
                nc.vector.scalar_tensor_tensor(
                    out=u0_buf[:], in0=h_buf[:], scalar=a0_col[:], in1=u0_buf[:],
                    op0=OP.mult, op1=OP.add)
                nc.vector.tensor_scalar_mul(u1_buf[:], x_buf[:], b1_col[:])
                nc.vector.scalar_tensor_tensor(
                    out=u1_buf[:], in0=h_buf[:], scalar=a1_col[:], in1=u1_buf[:],
                    op0=OP.mult, op1=OP.add)
                nc.vector.tensor_scalar_mul(xb2_buf[:], x_buf[:], d2_col[:])

            def epilogue_window(w):
                sl = slice(w * 128, (w + 1) * 128)
                h1_w = h1_buf[:, sl]

                u2 = wrkp.tile([128, 128], edt, tag="u2")
                nc.vector.scalar_tensor_tensor(
                    out=u2[:], in0=h1_w, scalar=c2_col[:], in1=xb2_buf[:, sl],
                    op0=OP.mult, op1=OP.subtract)

                oT = {}
                for nm, u in (("low", u0_buf[:, sl]), ("high", u1_buf[:, sl]),
                              ("mid", u2)):
                    up = psep.tile([128, 128], edt, space="PSUM", tag="uT")
                    nc.tensor.transpose(up[:], u, ident[:])
                    uT = wrkp.tile([128, 128], edt, tag=f"uT{nm}")
                    nc.vector.tensor_copy(uT[:], up[:])
                    op = psep.tile([128, 128], f32, space="PSUM", tag="om")
                    nc.tensor.matmul(op[:], lhsT=wT[nm][:], rhs=uT[:],
                                     start=True, stop=True)
                    ot = wrkp.tile([128, 128], edt, tag=f"ot{nm}")
                    nc.scalar.copy(ot[:], op[:])
                    oT[nm] = ot

                # mutual gating (T layout)
                tmp = wrkp.tile([128, 128], edt, tag="gt")
                sig = wrkp.tile([128, 128], edt, tag="gs")
                nc.vector.tensor_tensor(tmp[:], oT["high"][:], oT["mid"][:], OP.add)
                nc.scalar.activation(sig[:], tmp[:], AF.Sigmoid)
                nc.vector.tensor_tensor(oT["low"][:], oT["low"][:], sig[:], OP.mult)
                nc.vector.tensor_tensor(tmp[:], oT["low"][:], oT["mid"][:], OP.add)
                nc.scalar.activation(sig[:], tmp[:], AF.Sigmoid)
                nc.vector.tensor_tensor(oT["high"][:], oT["high"][:], sig[:], OP.mult)
                nc.vector.tensor_tensor(tmp[:], oT["low"][:], oT["high"][:], OP.add)
                nc.scalar.activation(sig[:], tmp[:], AF.Sigmoid)
                nc.vector.tensor_tensor(oT["mid"][:], oT["mid"][:], sig[:], OP.mult)

                nc.vector.tensor_tensor(tmp[:], oT["low"][:], oT["high"][:], OP.add)
                nc.vector.scalar_tensor_tensor(
                    out=tmp[:], in0=oT["mid"][:], scalar=bias_t[:], in1=tmp[:],
                    op0=OP.add, op1=OP.add)

                # back to row layout; relu(x * snorm)
                bp = psep.tile([128, 128], edt, space="PSUM", tag="uT")
                nc.tensor.transpose(bp[:], tmp[:], ident[:])
                outt = wrkp.tile([128, 128], f32, tag="outt")
                nc.scalar.activation(outt[:], bp[:], AF.Relu,
                                     scale=snorm_t[:, w:w + 1])
                nc.sync.dma_start(out_dram[w * 128:(w + 1) * 128, :], outt[:])

            def epilogue_window_scoped(w):
                with nc.named_scope("epi"):
                    epilogue_window(w)

            with nc.named_scope("mp1"):
                mp_round(1, h_buf, write_normh=True)
            with nc.named_scope("ag2"):
                nc.sync.dma_start(
                    bounce2[:].rearrange("(w p) d -> p w d", p=128),
                    nh_buf[:].rearrange("p (w d) -> p w d", d=D))
                nc.gpsimd.collective_compute(
                    "AllGather", mybir.AluOpType.bypass,
                    ins=[bounce2.opt()], outs=[ag2.opt()],
                    replica_groups=[list(range(NCORES))],
                )
            with nc.named_scope("epipre"):
                wide_epi_pre()
            if epi_interleave:
                with nc.named_scope("mp2"):
                    mp_round(2, h1_buf, write_normh=False,
                             window_done=epilogue_window_scoped)
            else:
                with nc.named_scope("mp2"):
                    mp_round(2, h1_buf, write_normh=False)
                for w in range(WINDOWS):
                    epilogue_window_scoped(w)

    nc.compile()
    return nc


# ---------------------------------------------------------------------------
# Public entry point
# ---------------------------------------------------------------------------

def build_and_inputs(feature, snorm_n, src, dst, W_low, W_high, W_mid,
                     gamma_low, gamma_high, gamma_mid, bias,
                     max_call=8, trim_pads=True, **build_kwargs):
    feature = np.asarray(feature, dtype=np.float32)
    snorm_n = np.asarray(snorm_n, dtype=np.float32)
    schedule, per_core = _preprocess(np.asarray(src), np.asarray(dst),
                                     max_call=max_call, trim_pads=trim_pads)
    nc = _build(schedule, **build_kwargs)

    feattab = np.zeros((AGROWS, D), np.float32)
    for c in range(NCORES):
        feattab[c * NPAD:c * NPAD + NPC] = feature[c * NPC:(c + 1) * NPC]

    in_maps = []
    for c in range(NCORES):
        feat_c = feattab[c * NPAD:(c + 1) * NPAD]
        sn_c = np.zeros(NPAD, np.float32)
        sn_c[:NPC] = snorm_n[c * NPC:(c + 1) * NPC, 0]
        in_maps.append({
            "feature": feat_c,
            "feattab": feattab,
            "snorm": sn_c.reshape(WINDOWS, 128).T.copy(),
            "deg": per_core[c]["deg"],
            "degsrc": per_core[c]["degsrc"],
            "dstoff": per_core[c]["dstoff"],
            "gidx": per_core[c]["gidx"],
            "W_lowT": np.ascontiguousarray(np.asarray(W_low, np.float32).T),
            "W_highT": np.ascontiguousarray(np.asarray(W_high, np.float32).T),
            "W_midT": np.ascontiguousarray(np.asarray(W_mid, np.float32).T),
            "gamma_low": np.asarray(gamma_low, np.float32).reshape(1, KG),
            "gamma_high": np.asarray(gamma_high, np.float32).reshape(1, KG),
            "gamma_mid": np.asarray(gamma_mid, np.float32).reshape(1, KG),
            "bias": np.asarray(bias, np.float32).reshape(128, 1),
        })

    return nc, in_maps


def kernel(**inputs):
    from concourse.bass_utils import run_bass_kernel_spmd

    nc, in_maps = build_and_inputs(**inputs)
    res = run_bass_kernel_spmd(nc, in_maps, core_ids=list(range(NCORES)))
    out = np.concatenate(
        [res.results[c]["out"][:NPC] for c in range(NCORES)], axis=0)
    return out
